# revision 17
# baseline (speedup 1.0000x reference)
import os
import sys

sys.path.insert(0, '/opt/trn_rl_repo')

import numpy as np
import ml_dtypes

import concourse.bass as bass
import concourse.mybir as mybir
import concourse.tile as tile
from concourse import bacc, bass_utils

B, T, S, C, NH, HD = 2, 32, 256, 1152, 16, 72
LY, MLP_H = 120, 4608
NCORES = 8
L = 2048            # tokens per core
KT = C // 128       # 9 K-tiles
CHUNK = 512
NCH = L // CHUNK    # 4
EPS = 1e-6
SCALE = HD ** -0.5

F32 = mybir.dt.float32
F32R = mybir.dt.float32r
BF16 = mybir.dt.bfloat16
AL = mybir.AluOpType
AF = mybir.ActivationFunctionType
BF = ml_dtypes.bfloat16

DUMP = bool(int(os.environ.get("KDUMP", "0")))

_built = {}


# ---------------------------------------------------------------- host prep

def _perm_p2o():
    # temporal q/k row permutation: position 128k+j <- orig row
    p = np.zeros(C, dtype=np.int64)
    for k in range(KT):
        for j in range(128):
            e = 64 * k + (j % 64)
            p[128 * k + j] = 2 * e + (0 if j < 64 else 1)
    return p


def _head_runs(h):
    # contiguous position-runs of head h's rows in the permuted layout,
    # ordered (even-comps in e-order, then odd-comps). Each run is
    # (dst_row0, pos0, n).
    runs = []
    dst = 0
    for par in range(2):
        a, bnd = 36 * h, 36 * h + 36
        pieces = []
        if a // 64 == (bnd - 1) // 64:
            pieces.append((a, bnd))
        else:
            mid = 64 * ((bnd - 1) // 64)
            pieces.append((a, mid))
            pieces.append((mid, bnd))
        for ea, eb in pieces:
            pos = 128 * (ea // 64) + (ea % 64) + 64 * par
            runs.append((dst, pos, eb - ea))
            dst += eb - ea
    return runs


def _host_prep(inputs):
    f32 = np.float32
    x = np.asarray(inputs['x'], f32)
    y = np.asarray(inputs['y'], f32)
    tvec = np.asarray(inputs['t'], f32)
    sst = np.asarray(inputs['scale_shift_table'], f32)

    ss = sst[None] + tvec.reshape(B, 6, C)      # (B, 6, C)
    sh_msa, sc_msa, g_msa, sh_mlp, sc_mlp, g_mlp = [ss[:, i] for i in range(6)]

    W = {k: np.asarray(v, f32) for k, v in inputs.items()}

    p2o = _perm_p2o()
    wqkv_t_T = W['temp_qkv_w'].T.copy()         # (C, 3C)
    wq = wqkv_t_T[:, :C][:, p2o]
    wk = wqkv_t_T[:, C:2 * C][:, p2o]
    wv = wqkv_t_T[:, 2 * C:]
    wqkv_t_T = np.concatenate([wq, wk, wv], axis=1)
    b_t = W['temp_qkv_b']
    b_qkt = np.concatenate([b_t[:C][p2o], b_t[C:2 * C][p2o]])   # (2C,)

    inv = 1.0 / (10000.0 ** (np.arange(0, HD, 2, dtype=f32) / HD))  # (36,)
    ang = np.outer(np.arange(T, dtype=f32), inv)                    # (32, 36)
    rows = np.arange(C)
    e_idx = 64 * (rows // 128) + (rows % 128) % 64
    i_idx = e_idx % 36
    cosR = np.cos(ang[:, i_idx]).T.astype(f32).copy()               # (1152, 32)
    sinR = np.sin(ang[:, i_idx]).T.astype(f32)
    sgn = np.where((rows % 128) < 64, -1.0, 1.0).astype(f32)
    sinS = (sinR * sgn[:, None]).copy()

    def dup2(v):            # (D,) -> (D, 2)
        return np.stack([v, v], axis=1).astype(f32)

    def per_b(m):           # (B, D) -> (D, B)
        return np.ascontiguousarray(m.T.astype(f32))

    b_qks_eff = per_b(sh_msa @ W['attn_qkv_w'].T[:, :2 * C]
                      + W['attn_qkv_b'][None, :2 * C])
    bv_s_eff = per_b(sh_msa @ W['attn_qkv_w'].T[:, 2 * C:]
                     + W['attn_qkv_b'][None, 2 * C:])
    b_fc1_eff = per_b(sh_mlp @ W['mlp_fc1_w'].T + W['mlp_fc1_b'][None])

    # temporal-attn mask for (t,si) packing: idx = t*4 + si
    tq = np.arange(128) // 4
    sq = np.arange(128) % 4
    mask = ((sq[:, None] == sq[None, :]) & (tq[None, :] >= tq[:, None]))
    mask_t = mask.astype(f32)

    shared = {
        'w_qkvs_T': W['attn_qkv_w'].T.copy().astype(BF),
        'b_qks': b_qks_eff,
        'bv_s': bv_s_eff,
        'w_projs_T': W['attn_proj_w'].T.copy().astype(BF),
        'g_msa': per_b(g_msa),
        'gb_projs': per_b(g_msa * W['attn_proj_b'][None]),
        'sc1p_msa': per_b(1.0 + sc_msa),
        'w_qkvt_T': wqkv_t_T.astype(BF),
        'b_qkt': dup2(b_qkt),
        'bv_t': dup2(b_t[2 * C:]),
        'cosR': cosR,
        'sinS': sinS,
        'w_projt_T': W['temp_proj_w'].T.copy().astype(BF),
        'gb_projt': per_b(g_msa * W['temp_proj_b'][None]),
        'w_qc_T': W['cross_q_w'].T.copy().astype(BF),
        'b_qc': per_b(np.stack([W['cross_q_b']] * B)),
        'w_kc_T': W['cross_kv_w'].T[:, :C].copy().astype(BF),
        'b_kc': dup2(W['cross_kv_b'][:C]),
        'w_vc_T': W['cross_kv_w'].T[:, C:].copy().astype(BF),
        'bv_c': dup2(W['cross_kv_b'][C:]),
        'w_projc_T': W['cross_proj_w'].T.copy().astype(BF),
        'b_projc': dup2(W['cross_proj_b']),
        'y_fm': np.ascontiguousarray(y.transpose(0, 2, 1)).astype(BF),
        'w_fc1_T': W['mlp_fc1_w'].T.copy().astype(BF),
        'b_fc1': b_fc1_eff,
        'w_fc2_T': W['mlp_fc2_w'].T.copy().astype(BF),
        'g_mlp': per_b(g_mlp),
        'gb_fc2': per_b(g_mlp * W['mlp_fc2_b'][None]),
        'sc1p_mlp': per_b(1.0 + sc_mlp),
        'mask_t': mask_t.astype(BF),
    }

    xr = x.reshape(B, T, S, C)
    in_maps = []
    for c in range(NCORES):
        xs = xr[:, 4 * c:4 * c + 4]                       # (2, 4, 256, C)
        x_fm = np.ascontiguousarray(xs.transpose(3, 0, 1, 2).reshape(C, L))
        m = dict(shared)
        m['x_fm'] = x_fm
        in_maps.append(m)
    return in_maps


def _host_gather(results):
    full = np.zeros((B, T, S, C), dtype=np.float32)
    for c in range(NCORES):
        o = results[c]['out_fm'].reshape(C, B, T, 32)
        full[:, :, 32 * c:32 * c + 32, :] = o.transpose(1, 2, 3, 0)
    return full.reshape(B, T * S, C)


# ---------------------------------------------------------------- builder

def build():
    nc = bacc.Bacc("TRN2", num_devices=NCORES, debug=False)

    def din(name, shape, dt):
        return nc.dram_tensor(name, shape, dt, kind="ExternalInput")

    x_fm = din('x_fm', (C, L), F32)
    w_qkvs_T = din('w_qkvs_T', (C, 3 * C), BF16)
    b_qks = din('b_qks', (2 * C, 2), F32)
    bv_s = din('bv_s', (C, 2), F32)
    w_projs_T = din('w_projs_T', (C, C), BF16)
    g_msa = din('g_msa', (C, 2), F32)
    gb_projs = din('gb_projs', (C, 2), F32)
    sc1p_msa = din('sc1p_msa', (C, 2), F32)
    w_qkvt_T = din('w_qkvt_T', (C, 3 * C), BF16)
    b_qkt = din('b_qkt', (2 * C, 2), F32)
    bv_t = din('bv_t', (C, 2), F32)
    cosR_d = din('cosR', (C, T), F32)
    sinS_d = din('sinS', (C, T), F32)
    w_projt_T = din('w_projt_T', (C, C), BF16)
    gb_projt = din('gb_projt', (C, 2), F32)
    w_qc_T = din('w_qc_T', (C, C), BF16)
    b_qc = din('b_qc', (C, 2), F32)
    w_kc_T = din('w_kc_T', (C, C), BF16)
    b_kc = din('b_kc', (C, 2), F32)
    w_vc_T = din('w_vc_T', (C, C), BF16)
    bv_c = din('bv_c', (C, 2), F32)
    w_projc_T = din('w_projc_T', (C, C), BF16)
    b_projc = din('b_projc', (C, 2), F32)
    y_fm_d = din('y_fm', (B, C, LY), BF16)
    w_fc1_T = din('w_fc1_T', (C, MLP_H), BF16)
    b_fc1 = din('b_fc1', (MLP_H, 2), F32)
    w_fc2_T = din('w_fc2_T', (MLP_H, C), BF16)
    g_mlp = din('g_mlp', (C, 2), F32)
    gb_fc2 = din('gb_fc2', (C, 2), F32)
    sc1p_mlp = din('sc1p_mlp', (C, 2), F32)
    mask_t_d = din('mask_t', (128, 128), BF16)

    out_fm = nc.dram_tensor('out_fm', (C, L), F32, kind="ExternalOutput")

    dumps = {}

    def dump(name, shape, dt):
        if DUMP:
            dumps[name] = nc.dram_tensor(name, shape, dt, kind="ExternalOutput")
        return dumps.get(name)

    d_xmod = dump('d_xmod', (C, L), BF16)
    d_qks = dump('d_qks', (2 * C, L), BF16)
    d_vaugs = dump('d_vaugs', (L, C), BF16)
    d_os = dump('d_os', (C, L), BF16)
    d_binb = dump('d_binb', (8, C, 256), F32)
    d_x2 = dump('d_x2', (C, L), F32)
    d_qkt = dump('d_qkt', (2 * C, L), BF16)
    d_ot = dump('d_ot', (C, L), BF16)
    d_x3 = dump('d_x3', (C, L), F32)
    d_ky = dump('d_ky', (B, C, LY), BF16)
    d_vy = dump('d_vy', (B, LY, C), BF16)
    d_oc = dump('d_oc', (C, L), BF16)
    d_x4 = dump('d_x4', (C, L), F32)
    d_xm2 = dump('d_xm2', (C, L), BF16)

    with tile.TileContext(nc) as tc:
        dram = tc.alloc_tile_pool(name="dram", bufs=1, space="DRAM")
        cst = tc.alloc_tile_pool(name="cst", bufs=1)

        qk_s = dram.tile([2 * C, L], BF16)
        v_s = dram.tile([L, C], BF16)
        o_s = dram.tile([C, L], BF16)
        bounce_in = dram.tile([8, C, 256], F32)
        bounce_out = dram.tile([8, C, 256], F32)
        x2 = dram.tile([C, L], F32)
        qk_t = dram.tile([2 * C, L], BF16)
        v_t = dram.tile([L, C], BF16)
        o_t = dram.tile([C, L], F32 if False else BF16)
        x3 = dram.tile([C, L], F32)
        q_c = dram.tile([C, L], BF16)
        k_y = dram.tile([B, C, LY], BF16)
        v_y = dram.tile([B, LY, C], BF16)
        o_c = dram.tile([C, L], BF16)
        x4 = dram.tile([C, L], F32)
        h_mlp = dram.tile([MLP_H, L], BF16)

        # ------- persistent constants
        ones_col_bf = cst.tile([128, 1], BF16)
        nc.vector.memset(ones_col_bf[:], 1.0)
        ones_f = cst.tile([128, 1], F32)
        nc.vector.memset(ones_f[:], 1.0)
        ones_col_r = cst.tile([128, 1], F32R)
        nc.scalar.copy(ones_col_r[:], ones_f[:])
        ones_rowf = cst.tile([1, 128], F32)
        nc.vector.memset(ones_rowf[:], 1.0)
        ones_row_r = cst.tile([1, 128], F32R)
        nc.scalar.copy(ones_row_r[:], ones_rowf[:])
        eps_t = cst.tile([1, 1], F32)
        nc.vector.memset(eps_t[:], EPS)
        mask_sb = cst.tile([128, 128], BF16)
        nc.sync.dma_start(mask_sb[:], mask_t_d[:])

        def load_vec2(pool, src, ntiles, tag):
            ts = []
            for k in range(ntiles):
                t_ = pool.tile([128, 2], F32, tag=f"{tag}{k}")
                nc.sync.dma_start(t_[:], src[128 * k:128 * (k + 1), :])
                ts.append(t_)
            return ts


        # ---------------------------------------------------------- LN + mod
        def ln_mod(src_view, sc1p_src, xc_tiles, dump_t):
            # src_view: DRAM (C, L) f32; writes modulated bf16 into xc_tiles
            with (
                tc.tile_pool(name="lnf", bufs=KT + 2) as lnf,
                tc.tile_pool(name="lnt", bufs=3) as lnt,
                tc.tile_pool(name="lnr", bufs=2) as lnr,
                tc.tile_pool(name="lnp", bufs=2, space="PSUM") as lnp,
                tc.tile_pool(name="lnv", bufs=1) as lnv,
            ):
                sc1p_sb = load_vec2(lnv, sc1p_src, KT, "sc1p")
                for ch in range(NCH):
                    b = ch // 2
                    cs = slice(ch * CHUNK, (ch + 1) * CHUNK)
                    xf = []
                    for k in range(KT):
                        t_ = lnf.tile([128, CHUNK], F32, tag="xf")
                        nc.sync.dma_start(t_[:], src_view[128 * k:128 * (k + 1), cs])
                        xf.append(t_)
                    sum_ps = lnp.tile([1, CHUNK], F32, tag="sum")
                    ssq_ps = lnp.tile([1, CHUNK], F32, tag="ssq")
                    for k in range(KT):
                        xsq = lnt.tile([128, CHUNK], F32R, tag="xsq")
                        nc.scalar.square(xsq[:], xf[k][:])
                        xbf = lnt.tile([128, CHUNK], BF16, tag="xbf")
                        nc.scalar.copy(xbf[:], xf[k][:])
                        nc.tensor.matmul(sum_ps[:], lhsT=ones_col_bf[:], rhs=xbf[:],
                                         start=(k == 0), stop=(k == KT - 1))
                        nc.tensor.matmul(ssq_ps[:], lhsT=ones_col_r[:], rhs=xsq[:],
                                         start=(k == 0), stop=(k == KT - 1))
                    mean_r = lnr.tile([1, CHUNK], F32R, tag="mean")
                    with nc.allow_low_precision(reason="ln rows"):
                        nc.scalar.mul(mean_r[:], sum_ps[:], 1.0 / C)
                    msq_r = lnr.tile([1, CHUNK], F32, tag="msq")
                    nc.scalar.mul(msq_r[:], ssq_ps[:], 1.0 / C)
                    var_r = lnr.tile([1, CHUNK], F32, tag="var")
                    nc.vector.tensor_tensor(out=var_r[:], in0=mean_r[:],
                                            in1=mean_r[:], op=AL.mult)
                    nc.vector.tensor_tensor(out=var_r[:], in0=msq_r[:],
                                            in1=var_r[:], op=AL.subtract)
                    std_r = lnr.tile([1, CHUNK], F32, tag="std")
                    nc.scalar.activation(std_r[:], var_r[:], AF.Sqrt,
                                         bias=eps_t[:], scale=1.0)
                    rstd_r = lnr.tile([1, CHUNK], F32R, tag="rstd")
                    with nc.allow_low_precision(reason="ln rows"):
                        nc.vector.reciprocal(rstd_r[:], std_r[:])
                    mean_b = lnp.tile([128, CHUNK], F32, tag="meanb")
                    nc.tensor.matmul(mean_b[:], lhsT=ones_row_r[:], rhs=mean_r[:],
                                     start=True, stop=True)
                    rstd_b = lnp.tile([128, CHUNK], F32, tag="rstdb")
                    nc.tensor.matmul(rstd_b[:], lhsT=ones_row_r[:], rhs=rstd_r[:],
                                     start=True, stop=True)
                    for k in range(KT):
                        cen = lnt.tile([128, CHUNK], F32, tag="cen")
                        nc.vector.tensor_tensor(out=cen[:], in0=xf[k][:],
                                                in1=mean_b[:], op=AL.subtract)
                        with nc.allow_low_precision(reason="bf16 out"):
                            nc.vector.scalar_tensor_tensor(
                                out=xc_tiles[k][:, cs], in0=cen[:],
                                scalar=sc1p_sb[k][:, b:b + 1], in1=rstd_b[:],
                                op0=AL.mult, op1=AL.mult)
                if dump_t is not None:
                    for k in range(KT):
                        nc.sync.dma_start(dump_t[128 * k:128 * (k + 1), :],
                                          xc_tiles[k][:])

        # ---------------------------------------------------- fm projection
        def project_fm(xc_tiles, w_src, n_out, evict, wcol0=0, kt=KT):
            # out[ot*128+p, tok] = sum_K w_src[K, wcol0+ot*128+p] * xc[K, tok]
            with (
                tc.tile_pool(name="pw", bufs=2) as pw,
                tc.tile_pool(name="pp", bufs=4, space="PSUM") as pp,
            ):
                for ot in range(n_out // 128):
                    wt = []
                    for k in range(kt):
                        t_ = pw.tile([128, 128], BF16, tag=f"w{k}")
                        nc.sync.dma_start(
                            t_[:], w_src[128 * k:128 * (k + 1),
                                         wcol0 + 128 * ot:wcol0 + 128 * (ot + 1)])
                        wt.append(t_)
                    for ch in range(NCH):
                        cs = slice(ch * CHUNK, (ch + 1) * CHUNK)
                        ps = pp.tile([128, CHUNK], F32, tag="ps")
                        for k in range(kt):
                            nc.tensor.matmul(ps[:], lhsT=wt[k][:],
                                             rhs=xc_tiles[k][:, cs],
                                             start=(k == 0), stop=(k == kt - 1))
                        evict(ps, ot, ch)

        # ----------------------------------------- token-major v projection
        def project_v(xc_tiles, w_src, v_dst, dump_t):
            # v_dst[tok, of] = sum_K xc[K, tok] * w_src[K, 2C + of]
            with (
                tc.tile_pool(name="vw", bufs=2) as vw,
                tc.tile_pool(name="vp", bufs=4, space="PSUM") as vp,
                tc.tile_pool(name="ve", bufs=3) as ve,
            ):
                for hg in range(4):
                    wt = []
                    for k in range(KT):
                        t_ = vw.tile([128, 288], BF16, tag=f"w{k}")
                        nc.sync.dma_start(
                            t_[:], w_src[128 * k:128 * (k + 1),
                                         2 * C + 288 * hg:2 * C + 288 * (hg + 1)])
                        wt.append(t_)
                    for tt_ in range(L // 128):
                        ts_ = slice(tt_ * 128, (tt_ + 1) * 128)
                        ps = vp.tile([128, 288], F32, tag="ps")
                        for k in range(KT):
                            nc.tensor.matmul(ps[:], lhsT=xc_tiles[k][:, ts_],
                                             rhs=wt[k][:],
                                             start=(k == 0), stop=(k == KT - 1))
                        ev = ve.tile([128, 288], BF16, tag="ev")
                        nc.scalar.copy(ev[:], ps[:])
                        nc.sync.dma_start(
                            v_dst[ts_, 288 * hg:288 * (hg + 1)], ev[:])
            if dump_t is not None:
                nc.sync.dma_start(dump_t[:], v_dst[:])

        # ---------------------------------------------------------- phase 1+2
        xc_pool = tc.alloc_tile_pool(name="xmod", bufs=1)
        xmod = [xc_pool.tile([128, L], BF16, tag=f"xm{k}", name=f"xm{k}") for k in range(KT)]
        ln_mod(x_fm[:], sc1p_msa[:], xmod, d_xmod)

        with (
            tc.tile_pool(name="bq", bufs=1) as bq_pool,
            tc.tile_pool(name="eqk", bufs=3) as eqk,
        ):
            bqk_sb = load_vec2(bq_pool, b_qks[:], 2 * KT, "bqk")

            def ev_qks(ps, ot, ch):
                b = ch // 2
                sb = eqk.tile([128, CHUNK], BF16, tag="sb")
                nc.scalar.activation(sb[:], ps[:], AF.Identity,
                                     bias=bqk_sb[ot][:, b:b + 1], scale=1.0)
                nc.sync.dma_start(
                    qk_s[128 * ot:128 * (ot + 1),
                         ch * CHUNK:(ch + 1) * CHUNK], sb[:])

            project_fm(xmod, w_qkvs_T[:], 2 * C, ev_qks)
        project_v(xmod, w_qkvs_T[:], v_s[:], d_vaugs)
        if d_qks is not None:
            nc.sync.dma_start(d_qks[:], qk_s[:])
        xc_pool.release()

        # ---------------------------------------------------------- phase 3
        def attn_generic(bv_src, n_iter_fh, q_ap_fn, v_ap_fn, o_ap_fn,
                         l_sz, m_tiles, m_sz, use_mask, name):
            with (
                tc.tile_pool(name=name + "a", bufs=3) as pa,
                tc.tile_pool(name=name + "p", bufs=2, space="PSUM") as pp,
                tc.tile_pool(name=name + "p2", bufs=2, space="PSUM") as pp2,
            ):
                for it in n_iter_fh:
                    b, h = it['b'], it['h']
                    q_sb = pa.tile([72, l_sz], BF16, tag="q")
                    for r0, src in q_ap_fn(it, 0):
                        nc.sync.dma_start(q_sb[r0:r0 + src.shape[0], :], src)
                    k_sb = pa.tile([72, m_tiles * m_sz], BF16, tag="k")
                    for r0, src in q_ap_fn(it, 1):
                        nc.sync.dma_start(k_sb[r0:r0 + src.shape[0], :], src)
                    oT = pp2.tile([72, l_sz], F32, tag="ot")
                    z_ps = pp2.tile([1, l_sz], F32, tag="z")
                    for mi in range(m_tiles):
                        ms = slice(mi * m_sz, (mi + 1) * m_sz)
                        e_ps = pp.tile([128, l_sz], F32, tag="e")
                        nc.tensor.matmul(e_ps[:m_sz], lhsT=k_sb[:, ms],
                                         rhs=q_sb[:], start=True, stop=True)
                        e_sb = pa.tile([128, l_sz], BF16, tag="es")
                        nc.scalar.activation(e_sb[:m_sz], e_ps[:m_sz], AF.Exp,
                                             bias=0.0, scale=SCALE)
                        if use_mask:
                            with nc.allow_low_precision(reason="mask"):
                                nc.vector.tensor_tensor(out=e_sb[:m_sz],
                                                        in0=e_sb[:m_sz],
                                                        in1=mask_sb[:m_sz, :l_sz],
                                                        op=AL.mult)
                        v_sb = pa.tile([m_sz, HD], BF16, tag="v")
                        nc.sync.dma_start(v_sb[:], v_ap_fn(it, mi))
                        nc.tensor.matmul(oT[:], lhsT=v_sb[:], rhs=e_sb[:m_sz],
                                         start=(mi == 0), stop=(mi == m_tiles - 1))
                        nc.tensor.matmul(z_ps[:], lhsT=ones_col_bf[:m_sz, :],
                                         rhs=e_sb[:m_sz],
                                         start=(mi == 0), stop=(mi == m_tiles - 1))
                    zr = pa.tile([1, l_sz], F32R, tag="zr")
                    with nc.allow_low_precision(reason="softmax z"):
                        nc.vector.reciprocal(zr[:], z_ps[:])
                    zb = pp.tile([72, l_sz], F32, tag="zb")
                    nc.tensor.matmul(zb[:], lhsT=ones_row_r[:, :72],
                                     rhs=zr[:], start=True, stop=True)
                    oc_sb = pa.tile([72, l_sz], F32, tag="ocs")
                    nc.scalar.copy(oc_sb[:], oT[:])
                    o1 = pa.tile([72, l_sz], F32, tag="o1")
                    nc.vector.tensor_tensor(out=o1[:], in0=oc_sb[:], in1=zb[:],
                                            op=AL.mult)
                    bvh = pa.tile([72, 1], F32, tag="bvh")
                    nc.sync.dma_start(bvh[:], bv_src[72 * h:72 * (h + 1), b:b + 1])
                    o_sb = pa.tile([72, l_sz], BF16, tag="ob")
                    nc.scalar.activation(o_sb[:], o1[:], AF.Identity,
                                         bias=bvh[:], scale=1.0)
                    nc.sync.dma_start(o_ap_fn(it), o_sb[:])

        # spatial attention: iterate (f, h); l=256, m=2x128
        sp_iters = [{'f': f, 'h': h, 'b': f // 4}
                    for f in range(8) for h in range(NH)]

        def sp_q(it, which):
            r0 = (0 if which == 0 else C) + 72 * it['h']
            return [(0, qk_s[r0:r0 + 72, 256 * it['f']:256 * (it['f'] + 1)])]

        def sp_v(it, mi):
            r0 = 256 * it['f'] + 128 * mi
            return v_s[r0:r0 + 128, 72 * it['h']:72 * (it['h'] + 1)]

        def sp_o(it):
            return o_s[72 * it['h']:72 * (it['h'] + 1),
                       256 * it['f']:256 * (it['f'] + 1)]

        attn_generic(bv_s[:], sp_iters, sp_q, sp_v, sp_o,
                     256, 2, 128, False, "sa")
        if d_os is not None:
            nc.sync.dma_start(d_os[:], o_s[:])

        # ---------------------------------------------------------- phase 4
        with (
            tc.tile_pool(name="p4v", bufs=1) as p4v,
            tc.tile_pool(name="p4o", bufs=3) as p4o,
            tc.tile_pool(name="p4c", bufs=1) as p4c,
        ):
            g_sb = load_vec2(p4v, g_msa[:], KT, "g")
            gb_sb = load_vec2(p4v, gb_projs[:], KT, "gb")
            os_c = [p4c.tile([128, L], BF16, tag=f"oc{k}", name=f"osc{k}") for k in range(KT)]
            for k in range(KT):
                nc.sync.dma_start(os_c[k][:], o_s[128 * k:128 * (k + 1), :])

            def ev_projs(ps, ot, ch):
                b = ch // 2
                tlh = ch % 2
                xo = p4o.tile([128, CHUNK], F32, tag="xo")
                nc.sync.dma_start(xo[:], x_fm[128 * ot:128 * (ot + 1),
                                              ch * CHUNK:(ch + 1) * CHUNK])
                s1 = p4o.tile([128, CHUNK], F32, tag="s1")
                nc.vector.scalar_tensor_tensor(out=s1[:], in0=ps[:],
                                               scalar=g_sb[ot][:, b:b + 1],
                                               in1=xo[:], op0=AL.mult, op1=AL.add)
                x1t = p4o.tile([128, 2, 256], F32, tag="x1t")
                nc.scalar.activation(x1t[:], s1[:].rearrange("p (a s) -> p a s", a=2),
                                     AF.Identity, bias=gb_sb[ot][:, b:b + 1],
                                     scale=1.0)
                for d in range(8):
                    nc.sync.dma_start(
                        bounce_in[d, 128 * ot:128 * (ot + 1),
                                  128 * b + 64 * tlh:128 * b + 64 * tlh + 64]
                        .rearrange("p (a s) -> p a s", a=2),
                        x1t[:, :, 32 * d:32 * d + 32])

            project_fm(os_c, w_projs_T[:], C, ev_projs)
        if d_binb is not None:
            nc.sync.dma_start(d_binb[:], bounce_in[:])

        # ---------------------------------------------------------- phase 5
        nc.gpsimd.collective_compute(
            "AllToAll", AL.bypass,
            ins=[bounce_in.opt()], outs=[bounce_out.opt()],
            replica_groups=[list(range(NCORES))],
        )
        for j in range(8):
            nc.sync.dma_start(
                x2[:].rearrange("r (b t sl) -> r b t sl", b=B, t=T)
                [:, :, 4 * j:4 * j + 4, :],
                bounce_out[j].rearrange("r (b tl sl) -> r b tl sl", b=B, tl=4))
        if d_x2 is not None:
            nc.sync.dma_start(d_x2[:], x2[:])

        # ---------------------------------------------------------- phase 6+7
        xc2_pool = tc.alloc_tile_pool(name="x2c", bufs=1)
        x2bf = [xc2_pool.tile([128, L], BF16, tag=f"x2b{k}", name=f"x2b{k}") for k in range(KT)]
        with tc.tile_pool(name="x2l", bufs=3) as x2l:
            for k in range(KT):
                for ch in range(NCH):
                    cs = slice(ch * CHUNK, (ch + 1) * CHUNK)
                    t_ = x2l.tile([128, CHUNK], F32, tag="x2f")
                    nc.sync.dma_start(t_[:], x2[128 * k:128 * (k + 1), cs])
                    nc.scalar.copy(x2bf[k][:, cs], t_[:])

        with tc.tile_pool(name="rope", bufs=1) as rp:
            cos_sb = []
            sin_sb = []
            for k in range(KT):
                ct_ = rp.tile([128, T], F32, tag=f"cos{k}")
                nc.sync.dma_start(ct_[:], cosR_d[128 * k:128 * (k + 1), :])
                cos_sb.append(ct_)
                st_ = rp.tile([128, T], F32, tag=f"sin{k}")
                nc.sync.dma_start(st_[:], sinS_d[128 * k:128 * (k + 1), :])
                sin_sb.append(st_)
            bqkt_sb = load_vec2(rp, b_qkt[:], 2 * KT, "bqkt")

            with tc.tile_pool(name="rpe", bufs=4) as rpe:
                def ev_qkt(ps, ot, ch):
                    k9 = ot % KT
                    # free pattern within chunk: (t 16, sl 32); chunk = (b, th)
                    th = ch % 2
                    s_sb = rpe.tile([128, CHUNK], F32, tag="s")
                    nc.scalar.activation(s_sb[:], ps[:], AF.Identity,
                                         bias=bqkt_sb[ot][:, 0:1], scale=1.0)
                    sw = rpe.tile([128, CHUNK], F32, tag="sw")
                    nc.sync.dma_start(sw[0:64, :], s_sb[64:128, :])
                    nc.sync.dma_start(sw[64:128, :], s_sb[0:64, :])
                    cos_ap = bass.AP(
                        tensor=cos_sb[k9].tensor,
                        offset=cos_sb[k9].offset + 16 * th,
                        ap=[cos_sb[k9].ap[0], [1, 16], [0, 32]])
                    sin_ap = bass.AP(
                        tensor=sin_sb[k9].tensor,
                        offset=sin_sb[k9].offset + 16 * th,
                        ap=[sin_sb[k9].ap[0], [1, 16], [0, 32]])
                    t1 = rpe.tile([128, 16, 32], F32, tag="t1")
                    nc.vector.tensor_tensor(
                        out=t1[:], in0=s_sb[:].rearrange("p (t sl) -> p t sl", t=16),
                        in1=cos_ap, op=AL.mult)
                    t2 = rpe.tile([128, 16, 32], F32, tag="t2")
                    nc.vector.tensor_tensor(
                        out=t2[:], in0=sw[:].rearrange("p (t sl) -> p t sl", t=16),
                        in1=sin_ap, op=AL.mult)
                    rot = rpe.tile([128, CHUNK], BF16, tag="rot")
                    with nc.allow_low_precision(reason="bf16 qk"):
                        nc.vector.tensor_tensor(
                            out=rot[:].rearrange("p (t sl) -> p t sl", t=16),
                            in0=t1[:], in1=t2[:], op=AL.add)
                    nc.sync.dma_start(
                        qk_t[128 * ot:128 * (ot + 1),
                             ch * CHUNK:(ch + 1) * CHUNK], rot[:])

                project_fm(x2bf, w_qkvt_T[:], 2 * C, ev_qkt)
        project_v(x2bf, w_qkvt_T[:], v_t[:], None)
        if d_qkt is not None:
            nc.sync.dma_start(d_qkt[:], qk_t[:])
        xc2_pool.release()

        # ---------------------------------------------------------- phase 8
        tp_iters = [{'b': b, 'h': h, 'sq': sq}
                    for b in range(B) for h in range(NH) for sq in range(8)]
        qkt_v = qk_t[:].rearrange("r (b t sq si) -> r b t sq si",
                                  b=B, t=T, sq=8)
        vt_v = v_t[:].rearrange("(b t sq si) c -> b t sq si c",
                                b=B, t=T, sq=8)
        ot_v = o_t[:].rearrange("r (b t sq si) -> r b t sq si",
                                b=B, t=T, sq=8)

        def tp_q(it, which):
            base = 0 if which == 0 else C
            out = []
            for dst, pos, n in _head_runs(it['h']):
                out.append((dst, qkt_v[base + pos:base + pos + n,
                                       it['b'], :, it['sq'], :]))
            return out

        def tp_v(it, mi):
            return vt_v[it['b'], :, it['sq'], :,
                        72 * it['h']:72 * (it['h'] + 1)]

        def tp_o(it):
            return ot_v[72 * it['h']:72 * (it['h'] + 1), it['b'], :, it['sq'], :]

        attn_generic(bv_t[:], tp_iters, tp_q, tp_v, tp_o,
                     128, 1, 128, True, "ta")
        if d_ot is not None:
            nc.sync.dma_start(d_ot[:], o_t[:])

        # ---------------------------------------------------------- phase 9
        x3bf_pool = tc.alloc_tile_pool(name="x3c", bufs=1)
        x3bf = [x3bf_pool.tile([128, L], BF16, tag=f"x3b{k}", name=f"x3b{k}")
                for k in range(KT)]
        with (
            tc.tile_pool(name="p9v", bufs=1) as p9v,
            tc.tile_pool(name="p9o", bufs=3) as p9o,
            tc.tile_pool(name="p9c", bufs=1) as p9c,
        ):
            g_sb = load_vec2(p9v, g_msa[:], KT, "g")
            gb_sb = load_vec2(p9v, gb_projt[:], KT, "gb")
            ot_c = [p9c.tile([128, L], BF16, tag=f"otc{k}", name=f"otc{k}") for k in range(KT)]
            for k in range(KT):
                nc.sync.dma_start(ot_c[k][:], o_t[128 * k:128 * (k + 1), :])

            def ev_projt(ps, ot, ch):
                b = ch // 2
                cs = slice(ch * CHUNK, (ch + 1) * CHUNK)
                xo = p9o.tile([128, CHUNK], F32, tag="xo")
                nc.sync.dma_start(xo[:], x2[128 * ot:128 * (ot + 1), cs])
                s1 = p9o.tile([128, CHUNK], F32, tag="s1")
                nc.vector.scalar_tensor_tensor(out=s1[:], in0=ps[:],
                                               scalar=g_sb[ot][:, b:b + 1],
                                               in1=xo[:], op0=AL.mult, op1=AL.add)
                x3t = p9o.tile([128, CHUNK], F32, tag="x3t")
                nc.scalar.activation(x3t[:], s1[:], AF.Identity,
                                     bias=gb_sb[ot][:, b:b + 1], scale=1.0)
                nc.sync.dma_start(x3[128 * ot:128 * (ot + 1), cs], x3t[:])
                with nc.allow_low_precision(reason="bf16 spine copy"):
                    nc.vector.tensor_copy(out=x3bf[ot][:, cs], in_=x3t[:])

            project_fm(ot_c, w_projt_T[:], C, ev_projt)
        if d_x3 is not None:
            nc.sync.dma_start(d_x3[:], x3[:])

        # ---------------------------------------------------------- phase 10
        # k_y / v_aug_y from y
        with (
            tc.tile_pool(name="yv", bufs=1) as yv,
            tc.tile_pool(name="yw", bufs=2) as yw,
            tc.tile_pool(name="yp", bufs=3, space="PSUM") as yp,
            tc.tile_pool(name="ye", bufs=3) as ye,
        ):
            y_sb = {}
            for b in range(B):
                for k in range(KT):
                    t_ = yv.tile([128, LY], BF16, tag=f"y{b}_{k}")
                    nc.sync.dma_start(t_[:], y_fm_d[b, 128 * k:128 * (k + 1), :])
                    y_sb[(b, k)] = t_
            bkc_sb = load_vec2(yv, b_kc[:], KT, "bkc")
            for b in range(B):
                for ot in range(KT):
                    ps = yp.tile([128, LY], F32, tag="kps")
                    for k in range(KT):
                        wt = yw.tile([128, 128], BF16, tag="kw")
                        nc.sync.dma_start(
                            wt[:], w_kc_T[128 * k:128 * (k + 1),
                                          128 * ot:128 * (ot + 1)])
                        nc.tensor.matmul(ps[:], lhsT=wt[:], rhs=y_sb[(b, k)][:],
                                         start=(k == 0), stop=(k == KT - 1))
                    sb = ye.tile([128, LY], BF16, tag="kev")
                    nc.scalar.activation(sb[:], ps[:], AF.Identity,
                                         bias=bkc_sb[ot][:, 0:1], scale=1.0)
                    nc.sync.dma_start(k_y[b, 128 * ot:128 * (ot + 1), :], sb[:])
                for hg in range(4):
                    ps = yp.tile([120, 288], F32, tag="vps")
                    for k in range(KT):
                        wt = yw.tile([128, 288], BF16, tag="vw")
                        nc.sync.dma_start(
                            wt[:], w_vc_T[128 * k:128 * (k + 1),
                                          288 * hg:288 * (hg + 1)])
                        nc.tensor.matmul(ps[:], lhsT=y_sb[(b, k)][:], rhs=wt[:],
                                         start=(k == 0), stop=(k == KT - 1))
                    sb = ye.tile([120, 288], BF16, tag="vev")
                    nc.scalar.copy(sb[:], ps[:])
                    nc.sync.dma_start(v_y[b, :, 288 * hg:288 * (hg + 1)], sb[:])
        if d_ky is not None:
            nc.sync.dma_start(d_ky[:], k_y[:])
            nc.sync.dma_start(d_vy[:], v_y[:])

        # q_c projection
        with tc.tile_pool(name="qcb", bufs=1) as qcb:
            bqc_sb = load_vec2(qcb, b_qc[:], KT, "bqc")

            def ev_qc(ps, ot, ch):
                b = ch // 2
                with tc.tile_pool(name="eqc", bufs=3) as eqc:
                    sb = eqc.tile([128, CHUNK], BF16, tag="sb")
                    nc.scalar.activation(sb[:], ps[:], AF.Identity,
                                         bias=bqc_sb[ot][:, b:b + 1], scale=1.0)
                    nc.sync.dma_start(
                        q_c[128 * ot:128 * (ot + 1),
                            ch * CHUNK:(ch + 1) * CHUNK], sb[:])

            project_fm(x3bf, w_qc_T[:], C, ev_qc)

        # cross attention: (b, h, c2); l=512, m=120 single tile
        cr_iters = [{'b': b, 'h': h, 'c2': c2}
                    for b in range(B) for h in range(NH) for c2 in range(2)]

        def cr_q(it, which):
            if which == 0:
                c0 = 1024 * it['b'] + 512 * it['c2']
                return [(0, q_c[72 * it['h']:72 * (it['h'] + 1), c0:c0 + 512])]
            return [(0, k_y[it['b'], 72 * it['h']:72 * (it['h'] + 1), :])]

        def cr_v(it, mi):
            return v_y[it['b'], :, 72 * it['h']:72 * (it['h'] + 1)]

        def cr_o(it):
            c0 = 1024 * it['b'] + 512 * it['c2']
            return o_c[72 * it['h']:72 * (it['h'] + 1), c0:c0 + 512]

        attn_generic(bv_c[:], cr_iters, cr_q, cr_v, cr_o,
                     512, 1, 120, False, "ca")
        if d_oc is not None:
            nc.sync.dma_start(d_oc[:], o_c[:])
        x3bf_pool.release()

        # cross proj + residual (no gate)
        with (
            tc.tile_pool(name="pcv", bufs=1) as pcv,
            tc.tile_pool(name="pco", bufs=3) as pco,
            tc.tile_pool(name="pcc", bufs=1) as pcc,
        ):
            bpc_sb = load_vec2(pcv, b_projc[:], KT, "bpc")
            oc_c = [pcc.tile([128, L], BF16, tag=f"occ{k}", name=f"occ{k}") for k in range(KT)]
            for k in range(KT):
                nc.sync.dma_start(oc_c[k][:], o_c[128 * k:128 * (k + 1), :])

            def ev_projc(ps, ot, ch):
                cs = slice(ch * CHUNK, (ch + 1) * CHUNK)
                xo = pco.tile([128, CHUNK], F32, tag="xo")
                nc.sync.dma_start(xo[:], x3[128 * ot:128 * (ot + 1), cs])
                s1 = pco.tile([128, CHUNK], F32, tag="s1")
                nc.vector.scalar_tensor_tensor(out=s1[:], in0=ps[:],
                                               scalar=bpc_sb[ot][:, 0:1],
                                               in1=xo[:], op0=AL.add, op1=AL.add)
                nc.sync.dma_start(x4[128 * ot:128 * (ot + 1), cs], s1[:])

            project_fm(oc_c, w_projc_T[:], C, ev_projc)
        if d_x4 is not None:
            nc.sync.dma_start(d_x4[:], x4[:])

        # ---------------------------------------------------------- phase 11+12
        xm2_pool = tc.alloc_tile_pool(name="xm2", bufs=1)
        xm2 = [xm2_pool.tile([128, L], BF16, tag=f"m2{k}", name=f"m2{k}") for k in range(KT)]
        ln_mod(x4[:], sc1p_mlp[:], xm2, d_xm2)

        with tc.tile_pool(name="f1b", bufs=1) as f1b:
            bf1_sb = load_vec2(f1b, b_fc1[:], MLP_H // 128, "bf1")

            def ev_fc1(ps, ot, ch):
                b = ch // 2
                with tc.tile_pool(name="ef1", bufs=3) as ef1:
                    sb = ef1.tile([128, CHUNK], BF16, tag="sb")
                    nc.scalar.activation(sb[:], ps[:], AF.Gelu_apprx_tanh,
                                         bias=bf1_sb[ot][:, b:b + 1], scale=1.0)
                    nc.sync.dma_start(
                        h_mlp[128 * ot:128 * (ot + 1),
                              ch * CHUNK:(ch + 1) * CHUNK], sb[:])

            project_fm(xm2, w_fc1_T[:], MLP_H, ev_fc1)
        xm2_pool.release()

        # fc2: K = 4608 (36 tiles), process in 2 token-halves with h cached
        with (
            tc.tile_pool(name="f2v", bufs=1) as f2v,
            tc.tile_pool(name="f2o", bufs=3) as f2o,
        ):
            g2_sb = load_vec2(f2v, g_mlp[:], KT, "g2")
            gb2_sb = load_vec2(f2v, gb_fc2[:], KT, "gb2")
            KT2 = MLP_H // 128
            for half in range(2):
                hs = slice(half * 1024, (half + 1) * 1024)
                with (
                    tc.tile_pool(name="f2h", bufs=1) as f2h,
                    tc.tile_pool(name="f2w", bufs=2) as f2w,
                    tc.tile_pool(name="f2p", bufs=4, space="PSUM") as f2p,
                ):
                    hc = []
                    for k in range(KT2):
                        t_ = f2h.tile([128, 1024], BF16, tag=f"h{k}")
                        nc.sync.dma_start(t_[:], h_mlp[128 * k:128 * (k + 1), hs])
                        hc.append(t_)
                    for ot in range(KT):
                        wt = []
                        for k in range(KT2):
                            t_ = f2w.tile([128, 128], BF16, tag=f"w{k}")
                            nc.sync.dma_start(
                                t_[:], w_fc2_T[128 * k:128 * (k + 1),
                                               128 * ot:128 * (ot + 1)])
                            wt.append(t_)
                        for ci in range(2):
                            ch = 2 * half + ci
                            b = ch // 2
                            cs = slice(ch * CHUNK, (ch + 1) * CHUNK)
                            ps = f2p.tile([128, CHUNK], F32, tag="ps")
                            for k in range(KT2):
                                nc.tensor.matmul(
                                    ps[:], lhsT=wt[k][:],
                                    rhs=hc[k][:, 512 * ci:512 * (ci + 1)],
                                    start=(k == 0), stop=(k == KT2 - 1))
                            xo = f2o.tile([128, CHUNK], F32, tag="xo")
                            nc.sync.dma_start(
                                xo[:], x4[128 * ot:128 * (ot + 1), cs])
                            s1 = f2o.tile([128, CHUNK], F32, tag="s1")
                            nc.vector.scalar_tensor_tensor(
                                out=s1[:], in0=ps[:],
                                scalar=g2_sb[ot][:, b:b + 1],
                                in1=xo[:], op0=AL.mult, op1=AL.add)
                            s2 = f2o.tile([128, CHUNK], F32, tag="s2")
                            nc.scalar.activation(s2[:], s1[:], AF.Identity,
                                                 bias=gb2_sb[ot][:, b:b + 1],
                                                 scale=1.0)
                            nc.sync.dma_start(
                                out_fm[128 * ot:128 * (ot + 1), cs], s2[:])

        cst.release()
        dram.release()

    nc.finalize()
    return nc


def get_nc():
    if 'nc' not in _built:
        _built['nc'] = build()
    return _built['nc']


def kernel(**inputs):
    nc = get_nc()
    in_maps = _host_prep(inputs)
    res = bass_utils.run_bass_kernel_spmd(
        nc, in_maps, core_ids=list(range(NCORES)))
    kernel.last_results = res
    return _host_gather(res.results)


# revision 28
# speedup vs baseline: 1.6745x; 1.6745x over previous
import os
import sys

sys.path.insert(0, '/opt/trn_rl_repo')

import numpy as np
import ml_dtypes

import concourse.bass as bass
import concourse.mybir as mybir
import concourse.tile as tile
from concourse import bacc, bass_utils

B, T, S, C, NH, HD = 2, 32, 256, 1152, 16, 72
LY, MLP_H = 120, 4608
NCORES = 8
L = 2048            # tokens per core
KT = C // 128       # 9 K-tiles
CHUNK = 512
NCH = L // CHUNK    # 4
EPS = 1e-6
SCALE = HD ** -0.5

F32 = mybir.dt.float32
F32R = mybir.dt.float32r
BF16 = mybir.dt.bfloat16
AL = mybir.AluOpType
AF = mybir.ActivationFunctionType
BF = ml_dtypes.bfloat16

DUMP = bool(int(os.environ.get("KDUMP", "0")))

_built = {}


# ---------------------------------------------------------------- host prep

def _perm_p2o():
    # temporal q/k row permutation: position 128k+j <- orig row
    p = np.zeros(C, dtype=np.int64)
    for k in range(KT):
        for j in range(128):
            e = 64 * k + (j % 64)
            p[128 * k + j] = 2 * e + (0 if j < 64 else 1)
    return p


def _head_runs(h):
    # contiguous position-runs of head h's rows in the permuted layout,
    # ordered (even-comps in e-order, then odd-comps). Each run is
    # (dst_row0, pos0, n).
    runs = []
    dst = 0
    for par in range(2):
        a, bnd = 36 * h, 36 * h + 36
        pieces = []
        if a // 64 == (bnd - 1) // 64:
            pieces.append((a, bnd))
        else:
            mid = 64 * ((bnd - 1) // 64)
            pieces.append((a, mid))
            pieces.append((mid, bnd))
        for ea, eb in pieces:
            pos = 128 * (ea // 64) + (ea % 64) + 64 * par
            runs.append((dst, pos, eb - ea))
            dst += eb - ea
    return runs


def _host_prep(inputs):
    f32 = np.float32
    x = np.asarray(inputs['x'], f32)
    y = np.asarray(inputs['y'], f32)
    tvec = np.asarray(inputs['t'], f32)
    sst = np.asarray(inputs['scale_shift_table'], f32)

    ss = sst[None] + tvec.reshape(B, 6, C)      # (B, 6, C)
    sh_msa, sc_msa, g_msa, sh_mlp, sc_mlp, g_mlp = [ss[:, i] for i in range(6)]

    W = {k: np.asarray(v, f32) for k, v in inputs.items()}

    p2o = _perm_p2o()
    wqkv_t_T = W['temp_qkv_w'].T.copy()         # (C, 3C)
    wq = wqkv_t_T[:, :C][:, p2o]
    wk = wqkv_t_T[:, C:2 * C][:, p2o]
    wv = wqkv_t_T[:, 2 * C:]
    wqkv_t_T = np.concatenate([wq, wk, wv], axis=1)
    b_t = W['temp_qkv_b']
    b_qkt = np.concatenate([b_t[:C][p2o], b_t[C:2 * C][p2o]])   # (2C,)

    inv = 1.0 / (10000.0 ** (np.arange(0, HD, 2, dtype=f32) / HD))  # (36,)
    ang = np.outer(np.arange(T, dtype=f32), inv)                    # (32, 36)
    rows = np.arange(C)
    e_idx = 64 * (rows // 128) + (rows % 128) % 64
    i_idx = e_idx % 36
    cosR = np.cos(ang[:, i_idx]).T.astype(f32).copy()               # (1152, 32)
    sinR = np.sin(ang[:, i_idx]).T.astype(f32)
    sgn = np.where((rows % 128) < 64, -1.0, 1.0).astype(f32)
    sinS = (sinR * sgn[:, None]).copy()

    def dup2(v):            # (D,) -> (D, 2)
        return np.stack([v, v], axis=1).astype(f32)

    def headwise(m2):       # (C, 2) -> (72, NH, 2)
        return np.ascontiguousarray(
            m2.reshape(NH, HD, 2).transpose(1, 0, 2)).astype(f32)

    def per_b(m):           # (B, D) -> (D, B)
        return np.ascontiguousarray(m.T.astype(f32))

    b_qks_eff = per_b(sh_msa @ W['attn_qkv_w'].T[:, :2 * C]
                      + W['attn_qkv_b'][None, :2 * C])
    bv_s_eff = per_b(sh_msa @ W['attn_qkv_w'].T[:, 2 * C:]
                     + W['attn_qkv_b'][None, 2 * C:])
    b_fc1_eff = per_b(sh_mlp @ W['mlp_fc1_w'].T + W['mlp_fc1_b'][None])

    # temporal-attn mask for (t,si) packing: idx = t*4 + si
    tq = np.arange(128) // 4
    sq = np.arange(128) % 4
    mask = ((sq[:, None] == sq[None, :]) & (tq[None, :] >= tq[:, None]))
    mask_t = mask.astype(f32)

    shared = {
        'w_qkvs_T': W['attn_qkv_w'].T.copy().astype(BF),
        'b_qks': b_qks_eff,
        'bv_s': headwise(bv_s_eff),
        'w_projs_T': W['attn_proj_w'].T.copy().astype(BF),
        'g_msa': per_b(g_msa),
        'gb_projs': per_b(g_msa * W['attn_proj_b'][None]),
        'sc1p_msa': per_b(1.0 + sc_msa),
        'w_qkvt_T': wqkv_t_T.astype(BF),
        'b_qkt': dup2(b_qkt),
        'bv_t': headwise(dup2(b_t[2 * C:])),
        'cosR': cosR,
        'sinS': sinS,
        'w_projt_T': W['temp_proj_w'].T.copy().astype(BF),
        'gb_projt': per_b(g_msa * W['temp_proj_b'][None]),
        'w_qc_T': W['cross_q_w'].T.copy().astype(BF),
        'b_qc': per_b(np.stack([W['cross_q_b']] * B)),
        'w_kc_T': W['cross_kv_w'].T[:, :C].copy().astype(BF),
        'b_kc': dup2(W['cross_kv_b'][:C]),
        'w_vc_T': W['cross_kv_w'].T[:, C:].copy().astype(BF),
        'bv_c': headwise(dup2(W['cross_kv_b'][C:])),
        'w_projc_T': W['cross_proj_w'].T.copy().astype(BF),
        'b_projc': dup2(W['cross_proj_b']),
        'y_fm': np.ascontiguousarray(y.transpose(0, 2, 1)).astype(BF),
        'w_fc1_T': W['mlp_fc1_w'].T.copy().astype(BF),
        'b_fc1': b_fc1_eff,
        'w_fc2_T': W['mlp_fc2_w'].T.copy().astype(BF),
        'g_mlp': per_b(g_mlp),
        'gb_fc2': per_b(g_mlp * W['mlp_fc2_b'][None]),
        'sc1p_mlp': per_b(1.0 + sc_mlp),
        'mask_t': mask_t.astype(BF),
    }

    xr = x.reshape(B, T, S, C)
    in_maps = []
    for c in range(NCORES):
        xs = xr[:, 4 * c:4 * c + 4]                       # (2, 4, 256, C)
        x_fm = np.ascontiguousarray(xs.transpose(3, 0, 1, 2).reshape(C, L))
        m = dict(shared)
        m['x_fm'] = x_fm
        in_maps.append(m)
    return in_maps


def _host_gather(results):
    # out_fm cols are (b, sq 8, t 32, si 4); global s = 32*c + sq*4 + si
    full = np.zeros((B, T, S, C), dtype=np.float32)
    for c in range(NCORES):
        o = results[c]['out_fm'].reshape(C, B, 8, T, 4)
        o = o.transpose(1, 3, 2, 4, 0).reshape(B, T, 32, C)
        full[:, :, 32 * c:32 * c + 32, :] = o
    return full.reshape(B, T * S, C)


# ---------------------------------------------------------------- builder

def build():
    nc = bacc.Bacc("TRN2", num_devices=NCORES, debug=False)

    def din(name, shape, dt):
        return nc.dram_tensor(name, shape, dt, kind="ExternalInput")

    x_fm = din('x_fm', (C, L), F32)
    w_qkvs_T = din('w_qkvs_T', (C, 3 * C), BF16)
    b_qks = din('b_qks', (2 * C, 2), F32)
    bv_s = din('bv_s', (HD, NH, 2), F32)
    w_projs_T = din('w_projs_T', (C, C), BF16)
    g_msa = din('g_msa', (C, 2), F32)
    gb_projs = din('gb_projs', (C, 2), F32)
    sc1p_msa = din('sc1p_msa', (C, 2), F32)
    w_qkvt_T = din('w_qkvt_T', (C, 3 * C), BF16)
    b_qkt = din('b_qkt', (2 * C, 2), F32)
    bv_t = din('bv_t', (HD, NH, 2), F32)
    cosR_d = din('cosR', (C, T), F32)
    sinS_d = din('sinS', (C, T), F32)
    w_projt_T = din('w_projt_T', (C, C), BF16)
    gb_projt = din('gb_projt', (C, 2), F32)
    w_qc_T = din('w_qc_T', (C, C), BF16)
    b_qc = din('b_qc', (C, 2), F32)
    w_kc_T = din('w_kc_T', (C, C), BF16)
    b_kc = din('b_kc', (C, 2), F32)
    w_vc_T = din('w_vc_T', (C, C), BF16)
    bv_c = din('bv_c', (HD, NH, 2), F32)
    w_projc_T = din('w_projc_T', (C, C), BF16)
    b_projc = din('b_projc', (C, 2), F32)
    y_fm_d = din('y_fm', (B, C, LY), BF16)
    w_fc1_T = din('w_fc1_T', (C, MLP_H), BF16)
    b_fc1 = din('b_fc1', (MLP_H, 2), F32)
    w_fc2_T = din('w_fc2_T', (MLP_H, C), BF16)
    g_mlp = din('g_mlp', (C, 2), F32)
    gb_fc2 = din('gb_fc2', (C, 2), F32)
    sc1p_mlp = din('sc1p_mlp', (C, 2), F32)
    mask_t_d = din('mask_t', (128, 128), BF16)

    out_fm = nc.dram_tensor('out_fm', (C, L), F32, kind="ExternalOutput")

    dumps = {}

    def dump(name, shape, dt):
        if DUMP:
            dumps[name] = nc.dram_tensor(name, shape, dt, kind="ExternalOutput")
        return dumps.get(name)

    d_xmod = dump('d_xmod', (C, L), BF16)
    d_qks = dump('d_qks', (2 * C, L), BF16)
    d_vaugs = dump('d_vaugs', (L, C), BF16)
    d_os = dump('d_os', (C, L), BF16)
    d_binb = dump('d_binb', (8, C, 256), F32)
    d_x2 = dump('d_x2', (C, L), F32)
    d_qkt = dump('d_qkt', (2 * C, L), BF16)
    d_ot = dump('d_ot', (C, L), BF16)
    d_x3 = dump('d_x3', (C, L), F32)
    d_ky = dump('d_ky', (B, C, LY), BF16)
    d_vy = dump('d_vy', (B, LY, C), BF16)
    d_oc = dump('d_oc', (C, L), BF16)
    d_x4 = dump('d_x4', (C, L), F32)
    d_xm2 = dump('d_xm2', (C, L), BF16)

    with tile.TileContext(nc) as tc:
        dram = tc.alloc_tile_pool(name="dram", bufs=1, space="DRAM")
        cst = tc.alloc_tile_pool(name="cst", bufs=1)

        qk_s = dram.tile([2 * C, L], BF16)
        v_s = dram.tile([L, C], BF16)
        o_s = dram.tile([C, L], BF16)
        x1 = dram.tile([C, L], F32)
        bounce_in = dram.tile([8, C, 256], F32)
        bounce_out = dram.tile([8, C, 256], F32)
        x2 = dram.tile([C, L], F32)
        qk_t = dram.tile([2 * C, L], BF16)
        v_t = dram.tile([L, C], BF16)
        o_t = dram.tile([C, L], F32 if False else BF16)
        x3 = dram.tile([C, L], F32)
        q_c = dram.tile([C, L], BF16)
        k_y = dram.tile([B, C, LY], BF16)
        v_y = dram.tile([B, LY, C], BF16)
        o_c = dram.tile([C, L], BF16)
        x4 = dram.tile([C, L], F32)
        h_mlp = dram.tile([MLP_H, L], BF16)

        # ------- persistent constants
        ones_col_bf = cst.tile([128, 1], BF16)
        nc.vector.memset(ones_col_bf[:], 1.0)
        ones_f = cst.tile([128, 1], F32)
        nc.vector.memset(ones_f[:], 1.0)
        ones_col_r = cst.tile([128, 1], F32R)
        nc.scalar.copy(ones_col_r[:], ones_f[:])
        ones_rowf = cst.tile([1, 128], F32)
        nc.vector.memset(ones_rowf[:], 1.0)
        ones_row_r = cst.tile([1, 128], F32R)
        nc.scalar.copy(ones_row_r[:], ones_rowf[:])
        eps_t = cst.tile([1, 1], F32)
        nc.vector.memset(eps_t[:], EPS)
        mask_sb = cst.tile([128, 128], BF16)
        nc.sync.dma_start(mask_sb[:], mask_t_d[:])

        def load_vec2(pool, src, ntiles, tag):
            ts = []
            for k in range(ntiles):
                t_ = pool.tile([128, 2], F32, tag=f"{tag}{k}")
                nc.sync.dma_start(t_[:], src[128 * k:128 * (k + 1), :])
                ts.append(t_)
            return ts


        # ---------------------------------------------------------- LN + mod
        def ln_mod(src_view, sc1p_src, xc_tiles, dump_t):
            # src_view: DRAM (C, L) f32; writes modulated bf16 into xc_tiles
            with (
                tc.tile_pool(name="lnf", bufs=KT + 2) as lnf,
                tc.tile_pool(name="lnt", bufs=3) as lnt,
                tc.tile_pool(name="lnr", bufs=2) as lnr,
                tc.tile_pool(name="lnp", bufs=2, space="PSUM") as lnp,
                tc.tile_pool(name="lnv", bufs=1) as lnv,
            ):
                sc1p_sb = load_vec2(lnv, sc1p_src, KT, "sc1p")
                for ch in range(NCH):
                    b = ch // 2
                    cs = slice(ch * CHUNK, (ch + 1) * CHUNK)
                    xf = []
                    for k in range(KT):
                        t_ = lnf.tile([128, CHUNK], F32, tag="xf")
                        nc.sync.dma_start(t_[:], src_view[128 * k:128 * (k + 1), cs])
                        xf.append(t_)
                    sum_ps = lnp.tile([1, CHUNK], F32, tag="sum")
                    ssq_ps = lnp.tile([1, CHUNK], F32, tag="ssq")
                    for k in range(KT):
                        xsq = lnt.tile([128, CHUNK], F32R, tag="xsq")
                        nc.scalar.square(xsq[:], xf[k][:])
                        xbf = lnt.tile([128, CHUNK], BF16, tag="xbf")
                        nc.scalar.copy(xbf[:], xf[k][:])
                        nc.tensor.matmul(sum_ps[:], lhsT=ones_col_bf[:], rhs=xbf[:],
                                         start=(k == 0), stop=(k == KT - 1))
                        nc.tensor.matmul(ssq_ps[:], lhsT=ones_col_r[:], rhs=xsq[:],
                                         start=(k == 0), stop=(k == KT - 1))
                    mean_r = lnr.tile([1, CHUNK], F32R, tag="mean")
                    with nc.allow_low_precision(reason="ln rows"):
                        nc.scalar.mul(mean_r[:], sum_ps[:], 1.0 / C)
                    msq_r = lnr.tile([1, CHUNK], F32, tag="msq")
                    nc.scalar.mul(msq_r[:], ssq_ps[:], 1.0 / C)
                    var_r = lnr.tile([1, CHUNK], F32, tag="var")
                    nc.vector.tensor_tensor(out=var_r[:], in0=mean_r[:],
                                            in1=mean_r[:], op=AL.mult)
                    nc.vector.tensor_tensor(out=var_r[:], in0=msq_r[:],
                                            in1=var_r[:], op=AL.subtract)
                    std_r = lnr.tile([1, CHUNK], F32, tag="std")
                    nc.scalar.activation(std_r[:], var_r[:], AF.Sqrt,
                                         bias=eps_t[:], scale=1.0)
                    rstd_r = lnr.tile([1, CHUNK], F32R, tag="rstd")
                    with nc.allow_low_precision(reason="ln rows"):
                        nc.vector.reciprocal(rstd_r[:], std_r[:])
                    mean_b = lnp.tile([128, CHUNK], F32, tag="meanb")
                    nc.tensor.matmul(mean_b[:], lhsT=ones_row_r[:], rhs=mean_r[:],
                                     start=True, stop=True)
                    rstd_b = lnp.tile([128, CHUNK], F32, tag="rstdb")
                    nc.tensor.matmul(rstd_b[:], lhsT=ones_row_r[:], rhs=rstd_r[:],
                                     start=True, stop=True)
                    for k in range(KT):
                        cen = lnt.tile([128, CHUNK], F32, tag="cen")
                        nc.vector.tensor_tensor(out=cen[:], in0=xf[k][:],
                                                in1=mean_b[:], op=AL.subtract)
                        with nc.allow_low_precision(reason="bf16 out"):
                            nc.vector.scalar_tensor_tensor(
                                out=xc_tiles[k][:, cs], in0=cen[:],
                                scalar=sc1p_sb[k][:, b:b + 1], in1=rstd_b[:],
                                op0=AL.mult, op1=AL.mult)
                if dump_t is not None:
                    for k in range(KT):
                        nc.sync.dma_start(dump_t[128 * k:128 * (k + 1), :],
                                          xc_tiles[k][:])

        # ---------------------------------------------------- fm projection
        def project_fm(xc_tiles, w_src, n_out, evict, wcol0=0, kt=KT):
            # out[ot*128+p, tok] = sum_K w_src[K, wcol0+ot*128+p] * xc[K, tok]
            with (
                tc.tile_pool(name="pw", bufs=3) as pw,
                tc.tile_pool(name="pp", bufs=6, space="PSUM") as pp,
            ):
                w_view = w_src.rearrange("(kt p) m -> p kt m", p=128)
                for ot in range(n_out // 128):
                    wt = pw.tile([128, kt, 128], BF16, tag="w")
                    nc.sync.dma_start(
                        wt[:], w_view[:, :, wcol0 + 128 * ot:wcol0 + 128 * (ot + 1)])
                    for ch in range(NCH):
                        cs = slice(ch * CHUNK, (ch + 1) * CHUNK)
                        ps = pp.tile([128, CHUNK], F32, tag="ps")
                        for k in range(kt):
                            nc.tensor.matmul(ps[:], lhsT=wt[:, k, :],
                                             rhs=xc_tiles[k][:, cs],
                                             start=(k == 0), stop=(k == kt - 1))
                        evict(ps, ot, ch)

        # ----------------------------------------- token-major v projection
        def project_v(xc_tiles, w_src, v_dst, dump_t):
            # v_dst[tok, of] = sum_K xc[K, tok] * w_src[K, 2C + of]
            with (
                tc.tile_pool(name="vw", bufs=2) as vw,
                tc.tile_pool(name="vp", bufs=6, space="PSUM") as vp,
                tc.tile_pool(name="ve", bufs=3) as ve,
            ):
                w_view = w_src.rearrange("(kt p) m -> p kt m", p=128)
                for hg in range(4):
                    wt = vw.tile([128, KT, 288], BF16, tag="w")
                    nc.sync.dma_start(
                        wt[:], w_view[:, :, 2 * C + 288 * hg:2 * C + 288 * (hg + 1)])
                    for tt_ in range(L // 128):
                        ts_ = slice(tt_ * 128, (tt_ + 1) * 128)
                        ps = vp.tile([128, 288], F32, tag="ps")
                        for k in range(KT):
                            nc.tensor.matmul(ps[:], lhsT=xc_tiles[k][:, ts_],
                                             rhs=wt[:, k, :],
                                             start=(k == 0), stop=(k == KT - 1))
                        ev = ve.tile([128, 288], BF16, tag="ev")
                        nc.scalar.copy(ev[:], ps[:])
                        nc.gpsimd.dma_start(
                            v_dst[ts_, 288 * hg:288 * (hg + 1)], ev[:])
            if dump_t is not None:
                nc.sync.dma_start(dump_t[:], v_dst[:])

        # ---------------------------------------------------------- phase 1+2
        xc_pool = tc.alloc_tile_pool(name="xmod", bufs=1)
        xmod = [xc_pool.tile([128, L], BF16, tag=f"xm{k}", name=f"xm{k}") for k in range(KT)]
        ln_mod(x_fm[:], sc1p_msa[:], xmod, d_xmod)

        with (
            tc.tile_pool(name="bq", bufs=1) as bq_pool,
            tc.tile_pool(name="eqk", bufs=3) as eqk,
        ):
            bqk_sb = load_vec2(bq_pool, b_qks[:], 2 * KT, "bqk")

            def ev_qks(ps, ot, ch):
                b = ch // 2
                sb = eqk.tile([128, CHUNK], BF16, tag="sb")
                nc.scalar.activation(sb[:], ps[:], AF.Identity,
                                     bias=bqk_sb[ot][:, b:b + 1], scale=1.0)
                nc.gpsimd.dma_start(
                    qk_s[128 * ot:128 * (ot + 1),
                         ch * CHUNK:(ch + 1) * CHUNK], sb[:])

            project_fm(xmod, w_qkvs_T[:], 2 * C, ev_qks)
        project_v(xmod, w_qkvs_T[:], v_s[:], d_vaugs)
        if d_qks is not None:
            nc.sync.dma_start(d_qks[:], qk_s[:])
        xc_pool.release()

        # ---------------------------------------------------------- phase 3
        def softmax_block(pa, pp, oT, z_ps, l_sz, o_dst_ap, bv_ap):
            # oT/z accumulated in PSUM; normalize, add v-bias, write o_dst_ap
            zr = pa.tile([1, l_sz], F32R, tag="zr")
            with nc.allow_low_precision(reason="softmax z"):
                nc.vector.reciprocal(zr[:], z_ps[:])
            zb = pp.tile([72, l_sz], F32, tag="zb")
            nc.tensor.matmul(zb[:], lhsT=ones_row_r[:, :72], rhs=zr[:],
                             start=True, stop=True)
            oc_sb = pa.tile([72, l_sz], F32, tag="ocs")
            nc.scalar.copy(oc_sb[:], oT[:])
            o1 = pa.tile([72, l_sz], F32, tag="o1")
            nc.vector.tensor_tensor(out=o1[:], in0=oc_sb[:], in1=zb[:],
                                    op=AL.mult)
            nc.scalar.activation(o_dst_ap, o1[:], AF.Identity,
                                 bias=bv_ap, scale=1.0)

        # ---- spatial attention: h-outer, f-inner
        with (
            tc.tile_pool(name="sab", bufs=1) as sab,
            tc.tile_pool(name="saa", bufs=2) as saa,
            tc.tile_pool(name="sas", bufs=3) as sas,
            tc.tile_pool(name="sap", bufs=2, space="PSUM") as sapp,
            tc.tile_pool(name="sap2", bufs=2, space="PSUM") as sapp2,
        ):
            bvs_sb = sab.tile([HD, NH, 2], F32)
            nc.sync.dma_start(bvs_sb[:], bv_s[:])
            vsc = []
            for i in range(16):
                t_ = sab.tile([128, C], BF16, tag=f"vs{i}", name=f"vsc{i}")
                nc.gpsimd.dma_start(t_[:], v_s[128 * i:128 * (i + 1), :])
                vsc.append(t_)
            for h in range(NH):
                q_sb = saa.tile([72, L], BF16, tag="q")
                nc.sync.dma_start(q_sb[:], qk_s[72 * h:72 * h + 72, :])
                k_sb = saa.tile([72, L], BF16, tag="k")
                nc.sync.dma_start(k_sb[:], qk_s[C + 72 * h:C + 72 * h + 72, :])
                o_sb = saa.tile([72, L], BF16, tag="o")
                for f in range(8):
                    b = f // 4
                    fs = slice(256 * f, 256 * (f + 1))
                    oT = sapp2.tile([72, 256], F32, tag="ot")
                    z_ps = sapp2.tile([1, 256], F32, tag="z")
                    for mi in range(2):
                        ms = slice(256 * f + 128 * mi, 256 * f + 128 * (mi + 1))
                        e_ps = sapp.tile([128, 256], F32, tag="e")
                        nc.tensor.matmul(e_ps[:], lhsT=k_sb[:, ms],
                                         rhs=q_sb[:, fs], start=True, stop=True)
                        e_sb = sas.tile([128, 256], BF16, tag="es")
                        nc.scalar.activation(e_sb[:], e_ps[:], AF.Exp,
                                             bias=0.0, scale=SCALE)
                        v_ap = vsc[2 * f + mi][:, 72 * h:72 * h + 72]
                        nc.tensor.matmul(oT[:], lhsT=v_ap, rhs=e_sb[:],
                                         start=(mi == 0), stop=(mi == 1))
                        nc.tensor.matmul(z_ps[:], lhsT=ones_col_bf[:],
                                         rhs=e_sb[:],
                                         start=(mi == 0), stop=(mi == 1))
                    softmax_block(sas, sapp, oT, z_ps, 256, o_sb[:, fs],
                                  bvs_sb[:, h, b:b + 1])
                nc.gpsimd.dma_start(o_s[72 * h:72 * h + 72, :], o_sb[:])
        if d_os is not None:
            nc.sync.dma_start(d_os[:], o_s[:])

        # ---------------------------------------------------------- phase 4
        with (
            tc.tile_pool(name="p4v", bufs=1) as p4v,
            tc.tile_pool(name="p4o", bufs=3) as p4o,
            tc.tile_pool(name="p4c", bufs=1) as p4c,
        ):
            g_sb = load_vec2(p4v, g_msa[:], KT, "g")
            gb_sb = load_vec2(p4v, gb_projs[:], KT, "gb")
            os_c = [p4c.tile([128, L], BF16, tag=f"oc{k}", name=f"osc{k}") for k in range(KT)]
            for k in range(KT):
                nc.gpsimd.dma_start(os_c[k][:], o_s[128 * k:128 * (k + 1), :])

            def ev_projs(ps, ot, ch):
                b = ch // 2
                tlh = ch % 2
                xo = p4o.tile([128, CHUNK], F32, tag="xo")
                nc.sync.dma_start(xo[:], x_fm[128 * ot:128 * (ot + 1),
                                              ch * CHUNK:(ch + 1) * CHUNK])
                s1 = p4o.tile([128, CHUNK], F32, tag="s1")
                nc.vector.scalar_tensor_tensor(out=s1[:], in0=ps[:],
                                               scalar=g_sb[ot][:, b:b + 1],
                                               in1=xo[:], op0=AL.mult, op1=AL.add)
                x1t = p4o.tile([128, CHUNK], F32, tag="x1t")
                nc.scalar.activation(x1t[:], s1[:], AF.Identity,
                                     bias=gb_sb[ot][:, b:b + 1], scale=1.0)
                nc.sync.dma_start(x1[128 * ot:128 * (ot + 1),
                                     ch * CHUNK:(ch + 1) * CHUNK], x1t[:])

            project_fm(os_c, w_projs_T[:], C, ev_projs)
        x1v = x1[:].rearrange("r (b tl s) -> r b tl s", b=B, tl=4)
        for d in range(8):
            for b_ in range(B):
                nc.gpsimd.dma_start(
                    bounce_in[d, :, 128 * b_:128 * (b_ + 1)]
                    .rearrange("r (tl sl) -> r tl sl", tl=4),
                    x1v[:, b_, :, 32 * d:32 * d + 32])
        if d_binb is not None:
            nc.sync.dma_start(d_binb[:], bounce_in[:])

        # ---------------------------------------------------------- phase 5
        if os.environ.get("KNOCOLL") == "1":
            # timing-sim variant: collective replaced by local copy
            nc.sync.dma_start(bounce_out[:], bounce_in[:])
        else:
            nc.gpsimd.collective_compute(
                "AllToAll", AL.bypass,
                ins=[bounce_in.opt()], outs=[bounce_out.opt()],
                replica_groups=[list(range(NCORES))],
            )
        x2v = x2[:].rearrange("r (b sq t si) -> r b sq t si", b=B, sq=8, t=T)
        bov = bounce_out[:].rearrange("j r (b tl sl) -> j r b tl sl", b=B, tl=4)
        for j in range(8):
            for b_ in range(B):
                for tl in range(4):
                    nc.sync.dma_start(
                        x2v[:, b_, :, 4 * j + tl, :],
                        bov[j, :, b_, tl, :])
        if d_x2 is not None:
            nc.sync.dma_start(d_x2[:], x2[:])

        # ---------------------------------------------------------- phase 6+7
        xc2_pool = tc.alloc_tile_pool(name="x2c", bufs=1)
        x2bf = [xc2_pool.tile([128, L], BF16, tag=f"x2b{k}", name=f"x2b{k}") for k in range(KT)]
        with tc.tile_pool(name="x2l", bufs=3) as x2l:
            for k in range(KT):
                for ch in range(NCH):
                    cs = slice(ch * CHUNK, (ch + 1) * CHUNK)
                    t_ = x2l.tile([128, CHUNK], F32, tag="x2f")
                    nc.sync.dma_start(t_[:], x2[128 * k:128 * (k + 1), cs])
                    nc.scalar.copy(x2bf[k][:, cs], t_[:])

        with tc.tile_pool(name="rope", bufs=1) as rp:
            cos_sb = []
            sin_sb = []
            for k in range(KT):
                ct_ = rp.tile([128, T], F32, tag=f"cos{k}")
                nc.sync.dma_start(ct_[:], cosR_d[128 * k:128 * (k + 1), :])
                cos_sb.append(ct_)
                st_ = rp.tile([128, T], F32, tag=f"sin{k}")
                nc.sync.dma_start(st_[:], sinS_d[128 * k:128 * (k + 1), :])
                sin_sb.append(st_)
            bqkt_sb = load_vec2(rp, b_qkt[:], 2 * KT, "bqkt")

            with tc.tile_pool(name="rpe", bufs=4) as rpe:
                def ev_qkt(ps, ot, ch):
                    k9 = ot % KT
                    # chunk free pattern: (sq 4, t 32, si 4)
                    s_sb = rpe.tile([128, CHUNK], F32, tag="s")
                    nc.scalar.activation(s_sb[:], ps[:], AF.Identity,
                                         bias=bqkt_sb[ot][:, 0:1], scale=1.0)
                    sw = rpe.tile([128, CHUNK], F32, tag="sw")
                    nc.gpsimd.dma_start(sw[0:64, :], s_sb[64:128, :])
                    nc.gpsimd.dma_start(sw[64:128, :], s_sb[0:64, :])
                    cos_ap = bass.AP(
                        tensor=cos_sb[k9].tensor,
                        offset=cos_sb[k9].offset,
                        ap=[cos_sb[k9].ap[0], [0, 4], [1, 32], [0, 4]])
                    sin_ap = bass.AP(
                        tensor=sin_sb[k9].tensor,
                        offset=sin_sb[k9].offset,
                        ap=[sin_sb[k9].ap[0], [0, 4], [1, 32], [0, 4]])
                    t1 = rpe.tile([128, 4, 32, 4], F32, tag="t1")
                    nc.vector.tensor_tensor(
                        out=t1[:],
                        in0=s_sb[:].rearrange("p (sq t si) -> p sq t si", sq=4, t=32),
                        in1=cos_ap, op=AL.mult)
                    t2 = rpe.tile([128, 4, 32, 4], F32, tag="t2")
                    nc.vector.tensor_tensor(
                        out=t2[:],
                        in0=sw[:].rearrange("p (sq t si) -> p sq t si", sq=4, t=32),
                        in1=sin_ap, op=AL.mult)
                    rot = rpe.tile([128, CHUNK], BF16, tag="rot")
                    with nc.allow_low_precision(reason="bf16 qk"):
                        nc.vector.tensor_tensor(
                            out=rot[:].rearrange("p (sq t si) -> p sq t si",
                                                 sq=4, t=32),
                            in0=t1[:], in1=t2[:], op=AL.add)
                    nc.gpsimd.dma_start(
                        qk_t[128 * ot:128 * (ot + 1),
                             ch * CHUNK:(ch + 1) * CHUNK], rot[:])

                project_fm(x2bf, w_qkvt_T[:], 2 * C, ev_qkt)
        project_v(x2bf, w_qkvt_T[:], v_t[:], None)
        if d_qkt is not None:
            nc.sync.dma_start(d_qkt[:], qk_t[:])
        xc2_pool.release()

        # ---------------------------------------------------------- phase 8
        # token order: (b, sq, t, si); block (b,sq) = 128 consecutive tokens
        qkt_v = qk_t[:].rearrange("r (b sq t si) -> r b sq t si",
                                  b=B, sq=8, t=T)
        ot_v = o_t[:].rearrange("r (b sq t si) -> r b sq t si",
                                b=B, sq=8, t=T)
        with (
            tc.tile_pool(name="tab", bufs=1) as tab,
            tc.tile_pool(name="tvc", bufs=1) as tvc,
            tc.tile_pool(name="taa", bufs=2) as taa,
            tc.tile_pool(name="tas", bufs=3) as tas,
            tc.tile_pool(name="tap", bufs=2, space="PSUM") as tapp,
            tc.tile_pool(name="tap2", bufs=2, space="PSUM") as tapp2,
        ):
            bvt_sb = tab.tile([HD, NH, 2], F32)
            nc.sync.dma_start(bvt_sb[:], bv_t[:])
            vtc = []
            for i in range(16):
                t_ = tvc.tile([128, C], BF16, tag=f"vt{i}", name=f"vtc{i}")
                nc.gpsimd.dma_start(t_[:], v_t[128 * i:128 * (i + 1), :])
                vtc.append(t_)
            for b in range(B):
                for h in range(NH):
                    q_sb = taa.tile([72, 8, 128], BF16, tag="q")
                    k_sb = taa.tile([72, 8, 128], BF16, tag="k")
                    for r0, pos, n in _head_runs(h):
                        nc.sync.dma_start(q_sb[r0:r0 + n],
                                          qkt_v[pos:pos + n, b])
                        nc.sync.dma_start(k_sb[r0:r0 + n],
                                          qkt_v[C + pos:C + pos + n, b])
                    o_sb = taa.tile([72, 8, 128], BF16, tag="o")
                    for sq in range(8):
                        oT = tapp2.tile([72, 128], F32, tag="ot")
                        z_ps = tapp2.tile([1, 128], F32, tag="z")
                        e_ps = tapp.tile([128, 128], F32, tag="e")
                        nc.tensor.matmul(e_ps[:], lhsT=k_sb[:, sq, :],
                                         rhs=q_sb[:, sq, :],
                                         start=True, stop=True)
                        e_sb = tas.tile([128, 128], BF16, tag="es")
                        nc.scalar.activation(e_sb[:], e_ps[:], AF.Exp,
                                             bias=0.0, scale=SCALE)
                        with nc.allow_low_precision(reason="mask"):
                            nc.vector.tensor_tensor(out=e_sb[:], in0=e_sb[:],
                                                    in1=mask_sb[:], op=AL.mult)
                        v_ap = vtc[8 * b + sq][:, 72 * h:72 * h + 72]
                        nc.tensor.matmul(oT[:], lhsT=v_ap, rhs=e_sb[:],
                                         start=True, stop=True)
                        nc.tensor.matmul(z_ps[:], lhsT=ones_col_bf[:],
                                         rhs=e_sb[:], start=True, stop=True)
                        softmax_block(tas, tapp, oT, z_ps, 128,
                                      o_sb[:, sq, :], bvt_sb[:, h, b:b + 1])
                    nc.gpsimd.dma_start(ot_v[72 * h:72 * h + 72, b], o_sb[:])
        if d_ot is not None:
            nc.sync.dma_start(d_ot[:], o_t[:])

        # ---------------------------------------------------------- phase 9
        x3bf_pool = tc.alloc_tile_pool(name="x3c", bufs=1)
        x3bf = [x3bf_pool.tile([128, L], BF16, tag=f"x3b{k}", name=f"x3b{k}")
                for k in range(KT)]
        with (
            tc.tile_pool(name="p9v", bufs=1) as p9v,
            tc.tile_pool(name="p9o", bufs=3) as p9o,
            tc.tile_pool(name="p9c", bufs=1) as p9c,
        ):
            g_sb = load_vec2(p9v, g_msa[:], KT, "g")
            gb_sb = load_vec2(p9v, gb_projt[:], KT, "gb")
            ot_c = [p9c.tile([128, L], BF16, tag=f"otc{k}", name=f"otc{k}") for k in range(KT)]
            for k in range(KT):
                nc.gpsimd.dma_start(ot_c[k][:], o_t[128 * k:128 * (k + 1), :])

            def ev_projt(ps, ot, ch):
                b = ch // 2
                cs = slice(ch * CHUNK, (ch + 1) * CHUNK)
                xo = p9o.tile([128, CHUNK], F32, tag="xo")
                nc.sync.dma_start(xo[:], x2[128 * ot:128 * (ot + 1), cs])
                s1 = p9o.tile([128, CHUNK], F32, tag="s1")
                nc.vector.scalar_tensor_tensor(out=s1[:], in0=ps[:],
                                               scalar=g_sb[ot][:, b:b + 1],
                                               in1=xo[:], op0=AL.mult, op1=AL.add)
                x3t = p9o.tile([128, CHUNK], F32, tag="x3t")
                nc.scalar.activation(x3t[:], s1[:], AF.Identity,
                                     bias=gb_sb[ot][:, b:b + 1], scale=1.0)
                nc.gpsimd.dma_start(x3[128 * ot:128 * (ot + 1), cs], x3t[:])
                with nc.allow_low_precision(reason="bf16 spine copy"):
                    nc.vector.tensor_copy(out=x3bf[ot][:, cs], in_=x3t[:])

            project_fm(ot_c, w_projt_T[:], C, ev_projt)
        if d_x3 is not None:
            nc.sync.dma_start(d_x3[:], x3[:])

        # ---------------------------------------------------------- phase 10
        # k_y / v_aug_y from y
        with (
            tc.tile_pool(name="yv", bufs=1) as yv,
            tc.tile_pool(name="yw", bufs=2) as yw,
            tc.tile_pool(name="yp", bufs=3, space="PSUM") as yp,
            tc.tile_pool(name="ye", bufs=3) as ye,
        ):
            y_sb = {}
            for b in range(B):
                for k in range(KT):
                    t_ = yv.tile([128, LY], BF16, tag=f"y{b}_{k}")
                    nc.sync.dma_start(t_[:], y_fm_d[b, 128 * k:128 * (k + 1), :])
                    y_sb[(b, k)] = t_
            bkc_sb = load_vec2(yv, b_kc[:], KT, "bkc")
            wkc_v = w_kc_T[:].rearrange("(kt p) m -> p kt m", p=128)
            wvc_v = w_vc_T[:].rearrange("(kt p) m -> p kt m", p=128)
            for b in range(B):
                for ot in range(KT):
                    ps = yp.tile([128, LY], F32, tag="kps")
                    wt = yw.tile([128, KT, 128], BF16, tag="kw")
                    nc.sync.dma_start(wt[:],
                                      wkc_v[:, :, 128 * ot:128 * (ot + 1)])
                    for k in range(KT):
                        nc.tensor.matmul(ps[:], lhsT=wt[:, k, :],
                                         rhs=y_sb[(b, k)][:],
                                         start=(k == 0), stop=(k == KT - 1))
                    sb = ye.tile([128, LY], BF16, tag="kev")
                    nc.scalar.activation(sb[:], ps[:], AF.Identity,
                                         bias=bkc_sb[ot][:, 0:1], scale=1.0)
                    nc.gpsimd.dma_start(k_y[b, 128 * ot:128 * (ot + 1), :], sb[:])
                for hg in range(4):
                    ps = yp.tile([120, 288], F32, tag="vps")
                    wt = yw.tile([128, KT, 288], BF16, tag="vw")
                    nc.sync.dma_start(wt[:],
                                      wvc_v[:, :, 288 * hg:288 * (hg + 1)])
                    for k in range(KT):
                        nc.tensor.matmul(ps[:], lhsT=y_sb[(b, k)][:],
                                         rhs=wt[:, k, :],
                                         start=(k == 0), stop=(k == KT - 1))
                    sb = ye.tile([120, 288], BF16, tag="vev")
                    nc.scalar.copy(sb[:], ps[:])
                    nc.gpsimd.dma_start(v_y[b, :, 288 * hg:288 * (hg + 1)], sb[:])
        if d_ky is not None:
            nc.sync.dma_start(d_ky[:], k_y[:])
            nc.sync.dma_start(d_vy[:], v_y[:])

        # q_c projection
        with (
            tc.tile_pool(name="qcb", bufs=1) as qcb,
            tc.tile_pool(name="eqc", bufs=3) as eqc,
        ):
            bqc_sb = load_vec2(qcb, b_qc[:], KT, "bqc")

            def ev_qc(ps, ot, ch):
                b = ch // 2
                sb = eqc.tile([128, CHUNK], BF16, tag="sb")
                nc.scalar.activation(sb[:], ps[:], AF.Identity,
                                     bias=bqc_sb[ot][:, b:b + 1], scale=1.0)
                nc.gpsimd.dma_start(
                    q_c[128 * ot:128 * (ot + 1),
                        ch * CHUNK:(ch + 1) * CHUNK], sb[:])

            project_fm(x3bf, w_qc_T[:], C, ev_qc)

        # cross attention: (b,h)-outer
        with (
            tc.tile_pool(name="cab", bufs=1) as cab,
            tc.tile_pool(name="caa", bufs=2) as caa,
            tc.tile_pool(name="cas", bufs=3) as cas,
            tc.tile_pool(name="cap", bufs=2, space="PSUM") as capp,
            tc.tile_pool(name="cap2", bufs=2, space="PSUM") as capp2,
        ):
            bvc_sb = cab.tile([HD, NH, 2], F32)
            nc.sync.dma_start(bvc_sb[:], bv_c[:])
            vyc = []
            for b in range(B):
                t_ = cab.tile([LY, C], BF16, tag=f"vy{b}", name=f"vyc{b}")
                nc.gpsimd.dma_start(t_[:], v_y[b, :, :])
                vyc.append(t_)
            for b in range(B):
                for h in range(NH):
                    q_sb = caa.tile([72, 1024], BF16, tag="q")
                    nc.sync.dma_start(
                        q_sb[:], q_c[72 * h:72 * h + 72,
                                     1024 * b:1024 * (b + 1)])
                    k_sb = caa.tile([72, LY], BF16, tag="k")
                    nc.sync.dma_start(k_sb[:], k_y[b, 72 * h:72 * h + 72, :])
                    v_sb = vyc[b][:, 72 * h:72 * h + 72]
                    o_sb = caa.tile([72, 1024], BF16, tag="o")
                    for c2 in range(2):
                        cs2 = slice(512 * c2, 512 * (c2 + 1))
                        oT = capp2.tile([72, 512], F32, tag="ot")
                        z_ps = capp2.tile([1, 512], F32, tag="z")
                        e_ps = capp.tile([LY, 512], F32, tag="e")
                        nc.tensor.matmul(e_ps[:], lhsT=k_sb[:],
                                         rhs=q_sb[:, cs2], start=True, stop=True)
                        e_sb = cas.tile([LY, 512], BF16, tag="es")
                        nc.scalar.activation(e_sb[:], e_ps[:], AF.Exp,
                                             bias=0.0, scale=SCALE)
                        nc.tensor.matmul(oT[:], lhsT=v_sb, rhs=e_sb[:],
                                         start=True, stop=True)
                        nc.tensor.matmul(z_ps[:], lhsT=ones_col_bf[:LY, :],
                                         rhs=e_sb[:], start=True, stop=True)
                        softmax_block(cas, capp, oT, z_ps, 512,
                                      o_sb[:, cs2], bvc_sb[:, h, b:b + 1])
                    nc.gpsimd.dma_start(
                        o_c[72 * h:72 * h + 72, 1024 * b:1024 * (b + 1)], o_sb[:])
        if d_oc is not None:
            nc.sync.dma_start(d_oc[:], o_c[:])
        x3bf_pool.release()

        # cross proj + residual (no gate)
        with (
            tc.tile_pool(name="pcv", bufs=1) as pcv,
            tc.tile_pool(name="pco", bufs=3) as pco,
            tc.tile_pool(name="pcc", bufs=1) as pcc,
        ):
            bpc_sb = load_vec2(pcv, b_projc[:], KT, "bpc")
            oc_c = [pcc.tile([128, L], BF16, tag=f"occ{k}", name=f"occ{k}") for k in range(KT)]
            for k in range(KT):
                nc.gpsimd.dma_start(oc_c[k][:], o_c[128 * k:128 * (k + 1), :])

            def ev_projc(ps, ot, ch):
                cs = slice(ch * CHUNK, (ch + 1) * CHUNK)
                xo = pco.tile([128, CHUNK], F32, tag="xo")
                nc.sync.dma_start(xo[:], x3[128 * ot:128 * (ot + 1), cs])
                s1 = pco.tile([128, CHUNK], F32, tag="s1")
                nc.vector.scalar_tensor_tensor(out=s1[:], in0=ps[:],
                                               scalar=bpc_sb[ot][:, 0:1],
                                               in1=xo[:], op0=AL.add, op1=AL.add)
                nc.gpsimd.dma_start(x4[128 * ot:128 * (ot + 1), cs], s1[:])

            project_fm(oc_c, w_projc_T[:], C, ev_projc)
        if d_x4 is not None:
            nc.sync.dma_start(d_x4[:], x4[:])

        # ---------------------------------------------------------- phase 11+12
        xm2_pool = tc.alloc_tile_pool(name="xm2", bufs=1)
        xm2 = [xm2_pool.tile([128, L], BF16, tag=f"m2{k}", name=f"m2{k}") for k in range(KT)]
        ln_mod(x4[:], sc1p_mlp[:], xm2, d_xm2)

        with tc.tile_pool(name="f1b", bufs=1) as f1b:
            bf1_sb = load_vec2(f1b, b_fc1[:], MLP_H // 128, "bf1")

            def ev_fc1(ps, ot, ch):
                b = ch // 2
                with tc.tile_pool(name="ef1", bufs=3) as ef1:
                    sb = ef1.tile([128, CHUNK], BF16, tag="sb")
                    nc.scalar.activation(sb[:], ps[:], AF.Gelu_apprx_tanh,
                                         bias=bf1_sb[ot][:, b:b + 1], scale=1.0)
                    nc.gpsimd.dma_start(
                        h_mlp[128 * ot:128 * (ot + 1),
                              ch * CHUNK:(ch + 1) * CHUNK], sb[:])

            project_fm(xm2, w_fc1_T[:], MLP_H, ev_fc1)
        xm2_pool.release()

        # fc2: K = 4608 (36 tiles), process in 2 token-halves with h cached
        with (
            tc.tile_pool(name="f2v", bufs=1) as f2v,
            tc.tile_pool(name="f2o", bufs=3) as f2o,
        ):
            g2_sb = load_vec2(f2v, g_mlp[:], KT, "g2")
            gb2_sb = load_vec2(f2v, gb_fc2[:], KT, "gb2")
            KT2 = MLP_H // 128
            for half in range(2):
                hs = slice(half * 1024, (half + 1) * 1024)
                with (
                    tc.tile_pool(name="f2h", bufs=1) as f2h,
                    tc.tile_pool(name="f2w", bufs=2) as f2w,
                    tc.tile_pool(name="f2p", bufs=6, space="PSUM") as f2p,
                ):
                    hc = []
                    for k in range(KT2):
                        t_ = f2h.tile([128, 1024], BF16, tag=f"h{k}")
                        nc.gpsimd.dma_start(t_[:], h_mlp[128 * k:128 * (k + 1), hs])
                        hc.append(t_)
                    w2_v = w_fc2_T[:].rearrange("(kt p) m -> p kt m", p=128)
                    for ot in range(KT):
                        wt = f2w.tile([128, KT2, 128], BF16, tag="w")
                        nc.sync.dma_start(
                            wt[:], w2_v[:, :, 128 * ot:128 * (ot + 1)])
                        for ci in range(2):
                            ch = 2 * half + ci
                            b = ch // 2
                            cs = slice(ch * CHUNK, (ch + 1) * CHUNK)
                            ps = f2p.tile([128, CHUNK], F32, tag="ps")
                            for k in range(KT2):
                                nc.tensor.matmul(
                                    ps[:], lhsT=wt[:, k, :],
                                    rhs=hc[k][:, 512 * ci:512 * (ci + 1)],
                                    start=(k == 0), stop=(k == KT2 - 1))
                            xo = f2o.tile([128, CHUNK], F32, tag="xo")
                            nc.sync.dma_start(
                                xo[:], x4[128 * ot:128 * (ot + 1), cs])
                            s1 = f2o.tile([128, CHUNK], F32, tag="s1")
                            nc.vector.scalar_tensor_tensor(
                                out=s1[:], in0=ps[:],
                                scalar=g2_sb[ot][:, b:b + 1],
                                in1=xo[:], op0=AL.mult, op1=AL.add)
                            s2 = f2o.tile([128, CHUNK], F32, tag="s2")
                            nc.scalar.activation(s2[:], s1[:], AF.Identity,
                                                 bias=gb2_sb[ot][:, b:b + 1],
                                                 scale=1.0)
                            nc.gpsimd.dma_start(
                                out_fm[128 * ot:128 * (ot + 1), cs], s2[:])

        cst.release()
        dram.release()

    nc.finalize()
    return nc


def get_nc():
    if 'nc' not in _built:
        _built['nc'] = build()
    return _built['nc']


def kernel(**inputs):
    nc = get_nc()
    in_maps = _host_prep(inputs)
    res = bass_utils.run_bass_kernel_spmd(
        nc, in_maps, core_ids=list(range(NCORES)))
    kernel.last_results = res
    return _host_gather(res.results)
